# revision 33
# baseline (speedup 1.0000x reference)
"""ChannelTransposeAttention Trainium2 kernel.

Reference computation (B=4, C=64, H=W=64, N=H*W=4096):
  xn = LayerNorm_C(x) * g + b          (per-position norm over channels)
  q  = wq @ xn + bq                    (1x1 conv)
  k  = dwconv3x3(xn, wk) + bk
  v  = dwconv3x3(xn, wv) + bv
  attn = softmax_j(q^T k / sqrt(C))    (N x N spatial attention)
  out  = wo @ (attn @ v^T)^T + bo + x

Sharding: 8 cores = (batch b, i-half) pairs. Each core receives the full
batch image (for k/v), plus its own half of the rows (for q + residual),
and produces out[b, :, half] (64 x 2048).

Device pipeline per core:
  - LN via PE-transpose roundtrip (stats on DVE with bn_stats/bn_aggr)
  - depthwise conv as 9 accumulating diagonal matmuls on PE (f32r)
  - scores computed transposed: scoresT[j, i] = k_blk^T @ q  (f32r)
  - exp on ACT (scale=1/sqrt(C) fused), output bf16
  - AV matmul with ones-row-augmented v_t -> fused softmax denominators
  - normalize, wo-projection (bias via ones-row trick), residual add
"""

import numpy as np

import concourse.bacc as bacc
import concourse.bass as bass
import concourse.tile as tile
from concourse import bass_utils, mybir
from concourse.masks import make_identity

F32 = mybir.dt.float32
F32R = mybir.dt.float32r
BF16 = mybir.dt.bfloat16

B, C, H, W = 4, 64, 64, 64
N = H * W            # 4096
NQ = N // 2          # 2048 rows per core
HR = 72              # rolled+halo row count: [haloA,A(32),haloA,z,z,haloB,B(32),haloB,z,z]
WP = W + 2           # padded width (66)
ITILE = 1024         # i-chunk per attention pass
NPASS = NQ // ITILE  # 2
JBLK = N // 128      # 32 j-blocks
EXPF = mybir.ActivationFunctionType.Exp
LNF = mybir.ActivationFunctionType.Ln
SQUAREF = mybir.ActivationFunctionType.Square
SUB = mybir.AluOpType.subtract
MULT = mybir.AluOpType.mult
ADD = mybir.AluOpType.add

_CACHED_NC = None
CFG_EXP2048 = False  # one 2048-wide exp per score pair (4-bank psum slot)
CFG_DUALTAP = True   # contract tap pairs (dw=0,1) in K=128 via shifted half-1


def _pin_act_table_set():
    """Make Exp and Ln resolve to the single combined ACT table set.

    The act-table-load pass picks a set per activation; with Exp in
    exp_and_others and Ln in natural_log it reloads tables (~2.7us) on
    every LN/attention interleave point. Stripping Exp/Ln from all sets
    except natural_log_exp_and_others forces one load total. Set ids are
    positional, so only the contents are edited, never the dict order.
    """
    import functools

    import concourse.bacc as bacc_mod
    import concourse.hw_specs as hw_specs

    orig = hw_specs.get_activation_tables.__wrapped__

    @functools.cache
    def patched(module_arch):
        t = dict(orig(module_arch))
        both = {mybir.ActivationFunctionType.Exp, mybir.ActivationFunctionType.Ln}
        for name in t:
            if name != "natural_log_exp_and_others":
                t[name] = set(t[name]) - both
        return t

    hw_specs.get_activation_tables = patched
    for mod in (bacc_mod,):
        if hasattr(mod, "get_activation_tables"):
            mod.get_activation_tables = patched


def _build_nc(loop_iters=None):
    _pin_act_table_set()
    nc = bacc.Bacc(
        "TRN2",
        target_bir_lowering=False,
        debug=False,
        enable_asserts=False,
        num_devices=8,
    )
    d_xin = nc.dram_tensor("xin", (C, HR, W), F32, kind="ExternalInput").ap()
    d_wqT = nc.dram_tensor("wqT", (C, C), F32R, kind="ExternalInput").ap()
    d_bq = nc.dram_tensor("bq", (C, 1), F32, kind="ExternalInput").ap()
    d_woTb = nc.dram_tensor("woTb", (C + 1, C), F32R, kind="ExternalInput").ap()
    ntap = 6 if CFG_DUALTAP else 9
    d_kdiag = nc.dram_tensor(
        "kdiag", (2 * C, ntap, C), F32R, kind="ExternalInput"
    ).ap()
    d_vdiag = nc.dram_tensor(
        "vdiag", (2 * C, ntap, C), F32R, kind="ExternalInput"
    ).ap()
    d_bk = nc.dram_tensor("bk", (C, 1), F32, kind="ExternalInput").ap()
    d_bv = nc.dram_tensor("bv", (C, 1), F32, kind="ExternalInput").ap()
    d_y = nc.dram_tensor("y", (C, NQ), F32, kind="ExternalOutput").ap()

    with tile.TileContext(nc) as tc:
        _body(
            tc, d_xin, d_wqT, d_bq, d_woTb, d_kdiag, d_vdiag, d_bk, d_bv, d_y,
            loop_iters=loop_iters,
        )

    nc.compile()
    return nc


def _body(
    tc, d_xin, d_wqT, d_bq, d_woTb, d_kdiag, d_vdiag, d_bk, d_bv, d_y,
    loop_iters=None,
):
    nc = tc.nc
    ctx_pools = []

    def pool(name, bufs, space="SBUF"):
        p = tc.tile_pool(name=name, bufs=bufs, space=space)
        ctx_pools.append(p)
        return p.__enter__()

    const = pool("const", 1)
    big = pool("big", 1)
    work = pool("work", 8)
    epool = pool("epool", 3)
    tailp = pool("tailp", 2)
    # PSUM budget is 8 banks: scores 2x2 (or 1x4) + AV accumulator 2 + small 2x1.
    ps_s = pool("ps_s", 1 if CFG_EXP2048 else 2, space="PSUM")
    ps_av = pool("ps_av", 1, space="PSUM")
    ps1 = pool("ps1", 2, space="PSUM")

    # ---- constants / weights ----
    id64 = const.tile([64, 64], F32, tag="id64")
    make_identity(nc, id64)
    id128 = const.tile([128, 128], F32, tag="id128")
    make_identity(nc, id128)
    eps_t = const.tile([128, 1], F32, tag="eps")
    nc.vector.memset(eps_t, 1e-5)
    ones_row = const.tile([1, ITILE], F32, tag="ones_row")
    nc.vector.memset(ones_row, 1.0)
    zer72 = const.tile([2 * C, HR], F32, tag="zer72")
    nc.vector.memset(zer72, 0.0)
    # ---- big persistent buffers ----
    # The rolled+halo'd image: the core's own 32 rows (+halos) come first, so
    # fixed offsets select its q rows; DMA'd in LN-group-sized chunks so the
    # pipeline starts as soon as the first 512 positions land.
    xin_sb = big.tile([C, HR, W], F32, tag="xin_sb")
    for _i in range(HR // 8):
        nc.sync.dma_start(
            xin_sb[:, 8 * _i : 8 * (_i + 1), :], d_xin[:, 8 * _i : 8 * (_i + 1), :]
        )
    wqT = const.tile([C, C], F32R, tag="wqT")
    nc.sync.dma_start(wqT, d_wqT)
    bq_t = const.tile([C, 1], F32, tag="bq")
    nc.sync.dma_start(bq_t, d_bq)
    ntap = 6 if CFG_DUALTAP else 9
    kdiag = const.tile([2 * C, ntap, C], F32R, tag="kdiag")
    nc.sync.dma_start(kdiag, d_kdiag)
    vdiag = const.tile([2 * C, ntap, C], F32R, tag="vdiag")
    nc.sync.dma_start(vdiag, d_vdiag)
    woTb = const.tile([C + 1, C], F32R, tag="woTb")
    nc.sync.dma_start(woTb, d_woTb)
    bk = const.tile([C, 1], F32, tag="bk")
    nc.sync.dma_start(bk, d_bk)
    bv = const.tile([C, 1], F32, tag="bv")
    nc.sync.dma_start(bv, d_bv)
    # xn_pad is replicated on both partition halves so conv matmul pairs can
    # run concurrently in the two PE row-groups (tile_position row 0 / 64).
    # Rows 1:1 with xin_sb (halos are real LN'd rows); only W needs borders.
    xn_pad = big.tile([2 * C, HR, WP], F32R, tag="xn_pad")
    zcol = zer72.rearrange("p (a b) -> p a b", b=1)
    nc.vector.tensor_copy(xn_pad[:, :, 0:1], zcol)
    nc.vector.tensor_copy(xn_pad[:, :, WP - 1 : WP], zcol)
    if CFG_DUALTAP:
        nc.vector.tensor_copy(
            xn_pad[C : 2 * C, :, WP - 2 : WP - 1], zcol[C : 2 * C]
        )
    k_sb = big.tile([2 * C, N], F32R, tag="k_sb")
    v_sb = big.tile([C, N], F32, tag="v_sb")
    q_sb = big.tile([2 * C, NQ], F32R, tag="q_sb")
    vt_sb = big.tile([128, JBLK, C + 1], BF16, tag="vt_sb")
    nc.vector.memset(vt_sb[:, :, C : C + 1], 1.0)

    # ---- LayerNorm of a 512-position group (transpose roundtrip) ----
    # 4 PE transposes -> [128, 4, 64] psum; per-64-slice stats on DVE;
    # 4 PE transposes back -> [64, 512] psum; one strided copy out.
    def _b0(ap, n):
        """Broadcast the innermost free dim via a stride-0 AP."""
        return bass.AP(tensor=ap.tensor, offset=ap.offset, ap=[*ap.ap, [0, n]])

    def ln_group(src_ap, write_results):
        # forward tiles borrow the scores pool: 2-bank slots, idle during LN
        xt_ps = (ps1 if CFG_EXP2048 else ps_s).tile(
            [128, 4, 64], F32, tag="ps1" if CFG_EXP2048 else "ps_s"
        )
        for i in range(4):
            nc.tensor.transpose(
                xt_ps[:, i, :], src_ap[:, 128 * i : 128 * (i + 1)], id64
            )
        # stats with few, wide DVE ops (DVE drains make op count expensive):
        # sum/sumsq reductions + one fused var op; squares on ACT.
        sum4 = work.tile([128, 4], F32, tag="sum4")
        nc.vector.reduce_sum(sum4, xt_ps, axis=mybir.AxisListType.X)
        sq = work.tile([128, 4, 64], F32, tag="sq")
        nc.scalar.activation(sq, xt_ps, SQUAREF)
        sumsq4 = work.tile([128, 4], F32, tag="sumsq4")
        nc.vector.reduce_sum(sumsq4, sq, axis=mybir.AxisListType.X)
        mean4 = work.tile([128, 4], F32, tag="mean4")
        nc.vector.tensor_scalar_mul(mean4, sum4, 1.0 / 64.0)
        m2 = work.tile([128, 4], F32, tag="m2")
        nc.scalar.activation(m2, mean4, SQUAREF)
        var4 = work.tile([128, 4], F32, tag="var4")
        nc.vector.scalar_tensor_tensor(
            var4, sumsq4, 1.0 / 64.0, m2, op0=MULT, op1=SUB
        )
        # rstd = exp(-0.5*ln(var+eps)): Ln/Exp/Square share one ACT table set,
        # so interleaving with the attention exp causes no table reloads.
        rstd4 = work.tile([128, 4], F32, tag="rstd4")
        nc.scalar.activation(rstd4, var4, LNF, bias=eps_t)
        nc.scalar.activation(rstd4, rstd4, EXPF, scale=-0.5)
        xnt = work.tile([128, 4, 64], F32, tag="xnt")
        nc.vector.tensor_tensor(xnt, xt_ps, _b0(mean4[:], 64), op=SUB)
        nc.vector.tensor_tensor(xnt, xnt, _b0(rstd4[:], 64), op=MULT)
        tb_ps = ps1.tile([64, 4, 128], F32, tag="ps1")
        for i in range(4):
            nc.tensor.transpose(tb_ps[:, i, :], xnt[:, i, :], id128)
        write_results(tb_ps)

    xin_flat = xin_sb.rearrange("p a b -> p (a b)")

    def ln_full(t):
        """LN group t (8 rows of the rolled image) -> xn_pad rows 8t..8t+7."""
        def write_pad(tb_ps):
            src = tb_ps.rearrange("p a (c b) -> p (a c) b", b=W)
            c1 = 0 if CFG_DUALTAP else 1
            nc.vector.tensor_copy(xn_pad[0:C, 8 * t : 8 * t + 8, 1 : 1 + W], src)
            nc.vector.tensor_copy(
                xn_pad[C : 2 * C, 8 * t : 8 * t + 8, c1 : c1 + W], src
            )
        ln_group(xin_flat[:, 512 * t : 512 * (t + 1)], write_pad)

    # conv chunk table: chunk index -> (k/v column base, padded row base).
    # Chunks 0-3 are region A (core's own rows), 4-7 region B.
    CV_ROW = [1 + 8 * a for a in range(4)] + [37 + 8 * b for b in range(4)]

    def conv_pair(diag, bias_t, write_out, chn):
        """Depthwise 3x3 conv for chunks (chn, chn+1): chunk chn runs in PE
        row-group 0, chunk chn+1 concurrently in row-group 64."""
        psA = ps1.tile([64, 512], F32, tag="ps1")
        psB = ps1.tile([64, 512], F32, tag="ps1")
        if CFG_DUALTAP:
            # taps (dh,0)+(dh,1) contract together over K=128: half-1 of
            # xn_pad is stored one column left, so one dw=0 window reads
            # tap dw=0 from rows 0-63 and tap dw=1 from rows 64-127.
            for dh in range(3):
                for chx, cv_ps in ((chn, psA), (chn + 1, psB)):
                    r0 = CV_ROW[chx] - 1 + dh
                    nc.tensor.matmul(
                        cv_ps,
                        lhsT=diag[:, dh, :],
                        rhs=xn_pad[:, r0 : r0 + 8, 0:W],
                        start=(dh == 0), stop=False,
                    )
            for dh in range(3):  # dw=2 taps, chunk-paired in the row-groups
                for half, chx, cv_ps in ((0, chn, psA), (1, chn + 1, psB)):
                    r0 = CV_ROW[chx] - 1 + dh
                    dw0 = 2 - half  # half-1 storage is shifted one left
                    nc.tensor.matmul(
                        cv_ps,
                        lhsT=diag[half * C : (half + 1) * C, 3 + dh, :],
                        rhs=xn_pad[
                            half * C : (half + 1) * C, r0 : r0 + 8, dw0 : dw0 + W
                        ],
                        start=False, stop=(dh == 2),
                    )
        else:
            for tap in range(9):
                dh, dw = tap // 3, tap % 3
                for half, chx, cv_ps in ((0, chn, psA), (1, chn + 1, psB)):
                    r0 = CV_ROW[chx] - 1 + dh
                    rhs = xn_pad[
                        half * C : (half + 1) * C, r0 : r0 + 8, dw : dw + W
                    ]
                    nc.tensor.matmul(
                        cv_ps, lhsT=diag[half * C : (half + 1) * C, tap, :], rhs=rhs,
                        start=(tap == 0), stop=(tap == 8),
                    )
        write_out(chn, psA, bias_t)
        write_out(chn + 1, psB, bias_t)

    def write_k(chx, cv_ps, bias_t):
        nc.vector.tensor_scalar_add(
            k_sb[0:C, 512 * chx : 512 * (chx + 1)], cv_ps, bias_t
        )
        nc.vector.tensor_scalar_add(
            k_sb[C : 2 * C, 512 * chx : 512 * (chx + 1)], cv_ps, bias_t
        )

    def write_v(chx, cv_ps, bias_t):
        nc.vector.tensor_scalar_add(
            v_sb[:, 512 * chx : 512 * (chx + 1)], cv_ps, bias_t
        )

    def vt_make(jb):
        """Transposes for j-blocks (jb, jb+1), one batched copy out."""
        vt_ps = ps1.tile([128, 2, 64], F32, tag="ps1")
        nc.tensor.transpose(vt_ps[:, 0, :], v_sb[:, 128 * jb : 128 * (jb + 1)], id64)
        nc.tensor.transpose(
            vt_ps[:, 1, :], v_sb[:, 128 * (jb + 1) : 128 * (jb + 2)], id64
        )
        nc.vector.tensor_copy(vt_sb[:, jb : jb + 2, 0:C], vt_ps)

    def attn_pair(p, jb, av_ps):
        """Score blocks (jb, jb+1): jb in PE row-group 0, jb+1 in row-group
        64, concurrently; then exp + AV accumulate for both."""
        if CFG_EXP2048:
            sAB = ps_s.tile([128, 2, ITILE], F32, tag="ps_s")
            sA, sB = sAB[:, 0, :], sAB[:, 1, :]
        else:
            sA = ps_s.tile([128, ITILE], F32, tag="ps_s")
            sB = ps_s.tile([128, ITILE], F32, tag="ps_s")
        for ic in range(ITILE // 512):
            q0 = p * ITILE + 512 * ic
            nc.tensor.matmul(
                sA[:, 512 * ic : 512 * (ic + 1)],
                lhsT=k_sb[0:C, 128 * jb : 128 * (jb + 1)],
                rhs=q_sb[0:C, q0 : q0 + 512],
                start=True, stop=True,
            )
            nc.tensor.matmul(
                sB[:, 512 * ic : 512 * (ic + 1)],
                lhsT=k_sb[C : 2 * C, 128 * (jb + 1) : 128 * (jb + 2)],
                rhs=q_sb[C : 2 * C, q0 : q0 + 512],
                start=True, stop=True,
            )
        if CFG_EXP2048:
            e2 = epool.tile([128, 2, ITILE], BF16, tag="e_sb")
            nc.scalar.activation(e2, sAB, EXPF, scale=0.125)
            epairs = ((jb, e2[:, 0, :]), (jb + 1, e2[:, 1, :]))
        else:
            eA = epool.tile([128, ITILE], BF16, tag="e_sb")
            nc.scalar.activation(eA, sA, EXPF, scale=0.125)
            eB = epool.tile([128, ITILE], BF16, tag="e_sb")
            nc.scalar.activation(eB, sB, EXPF, scale=0.125)
            epairs = ((jb, eA), (jb + 1, eB))
        for jx, e_sb in epairs:
            for ic in range(ITILE // 512):
                nc.tensor.matmul(
                    av_ps[:, 512 * ic : 512 * (ic + 1)],
                    lhsT=vt_sb[:, jx, :],
                    rhs=e_sb[:, 512 * ic : 512 * (ic + 1)],
                    start=(jx == 0), stop=(jx == JBLK - 1),
                )

    def attn_tail(p, av_ps):
        r_sb = tailp.tile([1, ITILE], F32, tag="r_sb")
        nc.vector.reciprocal(r_sb, av_ps[C : C + 1, :])
        rb_sb = tailp.tile([64, ITILE], F32, tag="rb_sb")
        nc.gpsimd.partition_broadcast(rb_sb, r_sb)
        y1 = tailp.tile([C + 1, ITILE], F32R, tag="y1")
        nc.vector.tensor_copy(y1[C : C + 1, :], ones_row)
        nc.vector.tensor_tensor(y1[0:C, :], av_ps[0:C, :], rb_sb, op=MULT)
        for ic in range(ITILE // 512):
            o_ps = ps1.tile([64, 512], F32, tag="ps1")
            nc.tensor.matmul(
                o_ps, lhsT=woTb, rhs=y1[:, 512 * ic : 512 * (ic + 1)],
                start=True, stop=True,
            )
            yo = tailp.tile([64, 512], F32, tag="yo")
            r0 = 1 + 8 * (2 * p + ic)
            nc.vector.tensor_tensor(
                yo, o_ps,
                xin_sb[:, r0 : r0 + 8, :].rearrange("p a b -> p (a b)"),
                op=ADD,
            )
            nc.sync.dma_start(
                d_y[:, p * ITILE + 512 * ic : p * ITILE + 512 * (ic + 1)], yo
            )

    # ---- schedule ----
    # LN of the core's own rows first (gates the q projection), then the
    # pass-0 attention j-loop interleaved with full-image LN + conv chunks:
    # conv chunk ch feeds j-blocks 4ch..4ch+3, so exp (ACT) starts early and
    # conv matmuls fill PE slack while ACT is the per-jb bottleneck.
    def qproj(ic):
        qp = ps1.tile([64, 512], F32, tag="ps1")
        r0 = 1 + 8 * ic
        nc.tensor.matmul(
            qp, lhsT=wqT, rhs=xn_pad[0:C, r0 : r0 + 8, 1 : 1 + W],
            start=True, stop=True,
        )
        nc.vector.tensor_scalar_add(q_sb[0:C, 512 * ic : 512 * (ic + 1)], qp, bq_t)
        nc.vector.tensor_scalar_add(
            q_sb[C : 2 * C, 512 * ic : 512 * (ic + 1)], qp, bq_t
        )

    def schedule():
        # LN groups stream in row order; q and conv chunks consume them as
        # they land. Pass-0 attention starts after the first conv pair.
        ln_full(0)
        ln_full(1)
        qproj(0)
        ln_full(2)
        qproj(1)

        av0 = ps_av.tile([C + 1, ITILE], F32, tag="ps_av")
        LF_AT = {2: (3, 4), 4: (5, 6), 6: (7, 8)}
        Q_AT = {4: (2,), 6: (3,)}
        for ch in range(0, 8, 2):  # conv pair covers chunks ch, ch+1
            for t in LF_AT.get(ch, ()):
                ln_full(t)
            for ic in Q_AT.get(ch, ()):
                qproj(ic)
            conv_pair(kdiag, bk, write_k, ch)
            conv_pair(vdiag, bv, write_v, ch)
            for jb in range(8 * (ch // 2), 8 * (ch // 2) + 8, 2):
                vt_make(jb)
            for jb in range(8 * (ch // 2), 8 * (ch // 2) + 8, 2):
                attn_pair(0, jb, av0)
        attn_tail(0, av0)

        av1 = ps_av.tile([C + 1, ITILE], F32, tag="ps_av")
        for jb in range(0, JBLK, 2):
            attn_pair(1, jb, av1)
        attn_tail(1, av1)

    if loop_iters:
        with tc.For_i(0, loop_iters, 1):
            schedule()
    else:
        schedule()

    for p in reversed(ctx_pools):
        p.__exit__(None, None, None)


def _host_prep(x, ln_g, ln_b, wq, bq, wk, bk, wv, bv, wo, bo):
    """Fold LN affine into the projection weights; build per-core inputs."""
    f = np.float32
    g = np.asarray(ln_g, f)
    b = np.asarray(ln_b, f)
    wq = np.asarray(wq, f)
    wo = np.asarray(wo, f)
    wk = np.asarray(wk, f).reshape(C, 9)
    wv = np.asarray(wv, f).reshape(C, 9)

    wq_eff = wq * g[None, :]
    bq_eff = (np.asarray(bq, f) + wq @ b).reshape(C, 1)
    wqT = np.ascontiguousarray(wq_eff.T)
    woTb = np.concatenate([wo.T, np.asarray(bo, f)[None, :]], axis=0)

    wk_eff = wk * g[:, None]
    wv_eff = wv * g[:, None]
    bk_eff = (np.asarray(bk, f) + b * wk.sum(axis=1)).reshape(C, 1)
    bv_eff = (np.asarray(bv, f) + b * wv.sum(axis=1)).reshape(C, 1)

    idx = np.arange(C)
    from kernel import CFG_DUALTAP as _dual
    if _dual:
        # slots 0-2: duals (rows 0-63 tap (dh,0), rows 64-127 tap (dh,1));
        # slots 3-5: the dw=2 taps, replicated on both halves.
        kdiag = np.zeros((2 * C, 6, C), f)
        vdiag = np.zeros((2 * C, 6, C), f)
        for dh in range(3):
            kdiag[idx, dh, idx] = wk_eff[:, 3 * dh + 0]
            kdiag[C + idx, dh, idx] = wk_eff[:, 3 * dh + 1]
            vdiag[idx, dh, idx] = wv_eff[:, 3 * dh + 0]
            vdiag[C + idx, dh, idx] = wv_eff[:, 3 * dh + 1]
            kdiag[idx, 3 + dh, idx] = wk_eff[:, 3 * dh + 2]
            kdiag[C + idx, 3 + dh, idx] = wk_eff[:, 3 * dh + 2]
            vdiag[idx, 3 + dh, idx] = wv_eff[:, 3 * dh + 2]
            vdiag[C + idx, 3 + dh, idx] = wv_eff[:, 3 * dh + 2]
    else:
        kdiag = np.zeros((2 * C, 9, C), f)
        vdiag = np.zeros((2 * C, 9, C), f)
        for tap in range(9):
            kdiag[idx, tap, idx] = wk_eff[:, tap]
            kdiag[C + idx, tap, idx] = wk_eff[:, tap]
            vdiag[idx, tap, idx] = wv_eff[:, tap]
            vdiag[C + idx, tap, idx] = wv_eff[:, tap]

    x = np.asarray(x, f)
    in_maps = []
    for core in range(8):
        bb, half = core // 2, core % 2
        xb = x[bb]  # (C, H, W)
        xe = np.zeros((C, HR, W), f)
        a0 = half * 32           # region A: the core's own rows
        b0 = 32 - a0             # region B: the other half
        xe[:, 1:33, :] = xb[:, a0 : a0 + 32, :]
        if a0 > 0:
            xe[:, 0, :] = xb[:, a0 - 1, :]
        if a0 + 32 < H:
            xe[:, 33, :] = xb[:, a0 + 32, :]
        xe[:, 37:69, :] = xb[:, b0 : b0 + 32, :]
        if b0 > 0:
            xe[:, 36, :] = xb[:, b0 - 1, :]
        if b0 + 32 < H:
            xe[:, 69, :] = xb[:, b0 + 32, :]
        in_maps.append(
            {
                "xin": xe,
                "wqT": wqT,
                "bq": bq_eff,
                "woTb": woTb,
                "kdiag": kdiag,
                "vdiag": vdiag,
                "bk": bk_eff,
                "bv": bv_eff,
            }
        )
    return in_maps


def kernel(**inputs):
    global _CACHED_NC
    if _CACHED_NC is None:
        _CACHED_NC = _build_nc()
    nc = _CACHED_NC
    in_maps = _host_prep(**inputs)
    res = bass_utils.run_bass_kernel_spmd(nc, in_maps, core_ids=list(range(8)))
    out = np.empty((B, C, N), np.float32)
    for core in range(8):
        bb, half = core // 2, core % 2
        out[bb, :, half * NQ : (half + 1) * NQ] = res.results[core]["y"]
    return out.reshape(B, C, H, W)
